# revision 2
# baseline (speedup 1.0000x reference)
"""Trainium2 Bass kernel for nn_BaseLineModel (segment_reduce) — v3.

Gather-free design: the embedding lookup is computed as one-hot matmuls on
the tensor engine. Per 2048-token tile: DVE builds a [128, 2048] bf16
one-hot per 128-row vocab chunk (is_equal of int16 token ids against a
per-partition iota scalar), PE accumulates table_chunk^T @ onehot into a
[64, 2048] PSUM tile over all 235 chunks. The conv is 3 tap-shifted
64-contraction matmuls per 512-col group, maxpool on DVE.

Backend: the final Linear commutes with the segment mean, so each note is
projected to a scalar z_n = W0*delta_n + feats_n . W[1:] on PE, and the
segment reduction is a [notes -> 1024 days] one-hot matmul of [z, 1]
columns, ReduceScatter'd (8KB) across the 8 cores.
"""

import numpy as np
import ml_dtypes

import concourse.bass as bass
import concourse.mybir as mybir
import concourse.tile as tile
from concourse.bass_utils import run_bass_kernel_spmd
from concourse import library_config

# ---- problem dims (hardcoded per task contract) ----
N, L, E, H, K, V, S = 16384, 64, 64, 256, 3, 30000, 1024
NCORES = 8
NC_NOTES = N // NCORES            # 2048 notes per core
NTOK = NC_NOTES * L               # 131072 tokens per core
TILE = 2048                       # tokens per tile (32 notes)
NTILE = NTOK // TILE              # 64
NCHUNK = (V + 127) // 128         # 235 vocab chunks
VP = NCHUNK * 128                 # 30080
NOTES_PER_TILE = TILE // L        # 32
NDCHUNK = NC_NOTES // 128         # 16 note-chunks for segment phase

_SPLIT_MAXW = 1


def _split_waits(nc, maxw=_SPLIT_MAXW):
    """This walrus build rejects >1 sync wait per instruction; move extras
    onto preceding same-engine NOPs (sequencer order preserves semantics)."""
    for bb in nc.main_func.blocks:
        out = []
        for inst in bb.instructions:
            si = inst.sync_info
            waits = list(si.on_wait) if (si is not None and si.on_wait) else []
            if len(waits) > maxw:
                rest = waits[:-maxw]
                si.on_wait = waits[-maxw:]
                for i in range(0, len(rest), maxw):
                    out.append(mybir.InstNoOp(
                        name=f"{inst.name}-wsplit{i}",
                        sync_info=mybir.SyncInfo(on_wait=rest[i:i + maxw], on_update=[]),
                        bass_nofuse=True,
                        engine=inst.engine,
                    ))
            out.append(inst)
        bb.instructions = out


def _build_nc(reps=1, use_cc=True, mode='full'):
    f32 = mybir.dt.float32
    bf16 = mybir.dt.bfloat16
    i16 = mybir.dt.int16

    nc = bass.Bass()
    d_table = nc.declare_dram_parameter("table", [128, NCHUNK * 64], bf16, isOutput=False)
    d_tokb = nc.declare_dram_parameter("tokb", [64, NTOK // 64], f32, isOutput=False)
    d_iotab = nc.declare_dram_parameter("iotab", [128, NCHUNK], f32, isOutput=False)
    d_wk = nc.declare_dram_parameter("wk", [64, 768], bf16, isOutput=False)
    d_cb2 = nc.declare_dram_parameter("cb2", [128, 2], f32, isOutput=False)
    d_wz = nc.declare_dram_parameter("wz", [128, 2], f32, isOutput=False)
    d_dw0 = nc.declare_dram_parameter("dw0", [1, NC_NOTES], f32, isOutput=False)
    d_stf = nc.declare_dram_parameter("stf", [NC_NOTES, 1], f32, isOutput=False)
    d_iday = nc.declare_dram_parameter("iday", [128, S], f32, isOutput=False)
    d_bb = nc.declare_dram_parameter("bb", [128, 1], f32, isOutput=False)
    d_out = nc.declare_dram_parameter("out", [128, 1], f32, isOutput=True)
    part = nc.dram_tensor("part", [S, 2], f32)
    rs_out = nc.dram_tensor("rs_out", [128, 2], f32)

    with tile.TileContext(nc) as tc:
        nc.gpsimd.load_library(library_config.mlp)
        with (
            tc.tile_pool(name="cst", bufs=1) as cp,
            tc.tile_pool(name="feat", bufs=1) as fp,
        ):
         for _rep in range(reps):
            table_sb = cp.tile([128, NCHUNK * 64], bf16)
            nc.sync.dma_start(out=table_sb[:], in_=d_table[:])
            iotab_sb = cp.tile([128, NCHUNK], f32)
            nc.sync.dma_start(out=iotab_sb[:], in_=d_iotab[:])
            wk_sb = cp.tile([64, 768], bf16)
            nc.sync.dma_start(out=wk_sb[:], in_=d_wk[:])
            cb2_sb = cp.tile([128, 2], f32)
            nc.sync.dma_start(out=cb2_sb[:], in_=d_cb2[:])
            feats = [fp.tile([128, NC_NOTES], f32, name=f"feats{hh}_{_rep}")
                     for hh in range(2)]

            # ---- P1: one-hot gather + conv + maxpool, per 2048-token tile ----
            with (
                tc.tile_pool(name="tok", bufs=2) as tkp,
                tc.tile_pool(name="oh", bufs=4) as ohp,
                tc.tile_pool(name="xs", bufs=2) as xp,
                tc.tile_pool(name="xacc", bufs=1, space="PSUM") as xap,
                tc.tile_pool(name="ypsum", bufs=3, space="PSUM") as yp,
            ):
                for t in range(NTILE):
                    tok1 = tkp.tile([1, TILE], f32, tag="tok1")
                    nc.sync.dma_start(out=tok1[:], in_=d_tokb[t:t + 1, :])
                    tokrep = tkp.tile([128, TILE], f32, tag="tok")
                    nc.gpsimd.partition_broadcast(
                        out_ap=tokrep[:],
                        in_ap=tok1[:],
                    )
                    # 4 PSUM banks of [64, 512] (matmul out is bank-limited)
                    xaccs = [xap.tile([64, 512], f32, tag=f"xa{s}",
                                      name=f"xacc{t}_{s}") for s in range(4)]
                    for c in range(NCHUNK):
                        oh = ohp.tile([128, TILE], bf16, tag="oh")
                        nc.vector.tensor_scalar(
                            out=oh[:], in0=tokrep[:],
                            scalar1=iotab_sb[:, c:c + 1], scalar2=None,
                            op0=mybir.AluOpType.is_equal)
                        for s in range(4):
                            nc.tensor.matmul(out=xaccs[s][:],
                                             lhsT=table_sb[:, c * 64:(c + 1) * 64],
                                             rhs=oh[:, s * 512:(s + 1) * 512],
                                             start=(c == 0), stop=(c == NCHUNK - 1))
                    x_sb = xp.tile([64, TILE], bf16, tag="x")
                    for s in range(4):
                        nc.scalar.activation(out=x_sb[:, s * 512:(s + 1) * 512],
                                             in_=xaccs[s][:],
                                             func=mybir.ActivationFunctionType.Copy,
                                             scale=1.0)
                    # conv: per 512-col group, H-half: 3 tap-shifted matmuls
                    for g in range(4):
                        for hh in range(2):
                            y_ps = yp.tile([128, 512], f32, tag="y",
                                           name=f"y{t}_{g}_{hh}")
                            for k in range(K):
                                cols = min(512, TILE - g * 512 - k)
                                nc.tensor.matmul(
                                    out=y_ps[:, 0:cols],
                                    lhsT=wk_sb[:, (k * 2 + hh) * 128:(k * 2 + hh + 1) * 128],
                                    rhs=x_sb[:, g * 512 + k: g * 512 + k + cols],
                                    start=(k == 0), stop=(k == K - 1))
                            nc.vector.reduce_max(
                                out=feats[hh][:, t * NOTES_PER_TILE + g * 8:
                                              t * NOTES_PER_TILE + g * 8 + 8],
                                in_=y_ps[:].rearrange("p (n l) -> p n l", l=L)[:, :, 0:L - K + 1],
                                axis=mybir.AxisListType.X)

            # ---- P2: relu(feats + conv_b) ----
            for hh in range(2):
                nc.scalar.activation(out=feats[hh][:], in_=feats[hh][:],
                                     func=mybir.ActivationFunctionType.Relu,
                                     bias=cb2_sb[:, hh:hh + 1], scale=1.0)

            # ---- P3: per-note scalar z = feats . W[1:257] (+ W0*delta) ----
            with tc.tile_pool(name="zs", bufs=1) as zs:
                wz_sb = cp.tile([128, 2], f32)
                nc.sync.dma_start(out=wz_sb[:], in_=d_wz[:])
                dw0_sb = cp.tile([1, NC_NOTES], f32)
                nc.sync.dma_start(out=dw0_sb[:], in_=d_dw0[:])
                z_sb = zs.tile([1, NC_NOTES], f32)
                with tc.tile_pool(name="zp", bufs=2, space="PSUM") as zp:
                    for q in range(NC_NOTES // 512):
                        z_ps = zp.tile([1, 512], f32, tag="z", name=f"z{q}_{_rep}")
                        for hh in range(2):
                            nc.tensor.matmul(out=z_ps[:],
                                             lhsT=wz_sb[:, hh:hh + 1],
                                             rhs=feats[hh][:, q * 512:(q + 1) * 512],
                                             start=(hh == 0), stop=(hh == 1))
                        nc.vector.tensor_copy(out=z_sb[:, q * 512:(q + 1) * 512],
                                              in_=z_ps[:])
                nc.vector.tensor_add(out=z_sb[:], in0=z_sb[:], in1=dw0_sb[:])

                # ---- P4: z row -> 16 [128,1] columns via tiny matmuls ----
                ones11 = cp.tile([1, 1], f32)
                nc.vector.memset(ones11[:], 1.0)
                rhs16 = zs.tile([128, 32], f32)  # [z_col | ones] per note-chunk
                with tc.tile_pool(name="tp", bufs=2, space="PSUM") as tp:
                    for i in range(NDCHUNK):
                        zt_ps = tp.tile([128, 1], f32, tag="zt", name=f"zt{i}_{_rep}")
                        nc.tensor.matmul(out=zt_ps[:],
                                         lhsT=z_sb[0:1, i * 128:(i + 1) * 128],
                                         rhs=ones11[:],
                                         start=True, stop=True)
                        nc.vector.tensor_copy(out=rhs16[:, 2 * i:2 * i + 1],
                                              in_=zt_ps[:])
                        nc.vector.memset(rhs16[:, 2 * i + 1:2 * i + 2], 1.0)

                # ---- P5: segment-sum of [z, 1] over days via one-hot matmuls ----
                iday_sb = cp.tile([128, S], f32)
                nc.sync.dma_start(out=iday_sb[:], in_=d_iday[:])
                with (
                    tc.tile_pool(name="segsb", bufs=2) as ssp,
                    tc.tile_pool(name="segps", bufs=1, space="PSUM") as pp,
                ):
                    seg_ps = [pp.tile([128, 2], f32, tag=f"seg{bk}",
                                      name=f"seg{bk}_{_rep}") for bk in range(8)]
                    for i in range(NDCHUNK):
                        st_sb = ssp.tile([128, 1], f32, tag="st")
                        nc.sync.dma_start(out=st_sb[:],
                                          in_=d_stf[i * 128:(i + 1) * 128, :])
                        ohd = ssp.tile([128, S], f32, tag="ohd")
                        nc.vector.tensor_scalar(
                            out=ohd[:], in0=iday_sb[:],
                            scalar1=st_sb[:, 0:1], scalar2=None,
                            op0=mybir.AluOpType.is_equal)
                        for bk in range(8):
                            nc.tensor.matmul(out=seg_ps[bk][:],
                                             lhsT=ohd[:, bk * 128:(bk + 1) * 128],
                                             rhs=rhs16[:, 2 * i:2 * i + 2],
                                             start=(i == 0), stop=(i == NDCHUNK - 1))
                    for bk in range(8):
                        seg_sb = ssp.tile([128, 2], f32, tag="segout")
                        nc.vector.tensor_copy(out=seg_sb[:], in_=seg_ps[bk][:])
                        nc.sync.dma_start(out=part[bk * 128:(bk + 1) * 128, :],
                                          in_=seg_sb[:])

            # ---- P6: cross-core reduce + finalize ----
            if use_cc:
                with tc.tile_critical():
                    with nc.semaphore("cc_sem") as cc_sem:
                        nc.gpsimd.collective_compute(
                            "ReduceScatter", mybir.AluOpType.add,
                            replica_groups=[list(range(NCORES))],
                            ins=[part[:]], outs=[rs_out[:]],
                        ).then_inc(cc_sem, 1)
                        nc.gpsimd.wait_ge(cc_sem, 1)
            else:
                nc.sync.dma_start(out=rs_out[:], in_=part[0:128, :])

            with tc.tile_pool(name="fin", bufs=1) as fin:
                bb_sb = fin.tile([128, 1], f32)
                nc.sync.dma_start(out=bb_sb[:], in_=d_bb[:])
                fs = fin.tile([128, 2], f32)
                nc.sync.dma_start(out=fs[:], in_=rs_out[:])
                cnt = fin.tile([128, 1], f32)
                nc.vector.tensor_scalar_max(out=cnt[:], in0=fs[:, 1:2], scalar1=1.0)
                rcp = fin.tile([128, 1], f32)
                nc.vector.reciprocal(out=rcp[:], in_=cnt[:])
                dot = fin.tile([128, 1], f32)
                nc.vector.tensor_tensor(out=dot[:], in0=fs[:, 0:1], in1=rcp[:],
                                        op=mybir.AluOpType.mult)
                nc.vector.tensor_add(out=dot[:], in0=dot[:], in1=bb_sb[:])
                outsb = fin.tile([128, 1], f32)
                nc.scalar.activation(out=outsb[:], in_=dot[:],
                                     func=mybir.ActivationFunctionType.Sigmoid,
                                     scale=1.0)
                nc.sync.dma_start(out=d_out[:], in_=outsb[:])

    _split_waits(nc)
    mybir.codegen_inst_isa_subclasses(nc)
    return nc


_NC_CACHE = {}


def _get_nc(reps=1, use_cc=True, mode='full'):
    key = (reps, use_cc, mode)
    if key not in _NC_CACHE:
        _NC_CACHE[key] = _build_nc(reps, use_cc, mode)
    return _NC_CACHE[key]


def _prep_inputs(text, start_times, emb, conv_w, conv_b, W, b):
    bf16 = ml_dtypes.bfloat16
    text = np.asarray(text)[0]              # [N, L]
    st = np.asarray(start_times)[0].astype(np.int64)   # [N]
    emb = np.asarray(emb, dtype=np.float32)
    conv_w = np.asarray(conv_w, dtype=np.float32)
    conv_b = np.asarray(conv_b, dtype=np.float32)
    W = np.asarray(W, dtype=np.float32)
    b = np.asarray(b, dtype=np.float32)

    emb_pad = np.zeros((VP, E), np.float32)
    emb_pad[:V] = emb
    table = np.ascontiguousarray(
        emb_pad.reshape(NCHUNK, 128, E).transpose(1, 0, 2).reshape(128, NCHUNK * E)
    ).astype(bf16)

    iotab = (np.arange(NCHUNK, dtype=np.float32)[None, :] * 128
             + np.arange(128, dtype=np.float32)[:, None]).astype(np.float32)
    iotab = np.ascontiguousarray(iotab)

    # wk[e, (k*2+hh)*128 + h] = conv_w[hh*128+h, e, k]
    wk = np.zeros((E, 768), np.float32)
    for k in range(K):
        for hh in range(2):
            wk[:, (k * 2 + hh) * 128:(k * 2 + hh + 1) * 128] = \
                conv_w[hh * 128:(hh + 1) * 128, :, k].T
    wk = np.ascontiguousarray(wk.astype(bf16))

    cb2 = np.ascontiguousarray(conv_b.reshape(2, 128).T.astype(np.float32))
    wz = np.ascontiguousarray(
        W[1:H + 1, 0].reshape(2, 128).T.astype(np.float32))
    iday = np.tile(np.arange(S, dtype=np.float32), (128, 1))
    bbuf = np.full((128, 1), b[0], np.float32)

    delta_g = np.concatenate([[0.0], np.diff(st).astype(np.float32)]).astype(np.float32)

    tok = text.astype(np.float32)           # ids exact in f32
    in_maps = []
    for c in range(NCORES):
        sl = slice(c * NC_NOTES, (c + 1) * NC_NOTES)
        tokb = np.ascontiguousarray(tok[sl].reshape(64, NTOK // 64))
        in_maps.append({
            "table": table,
            "tokb": tokb,
            "iotab": iotab,
            "wk": wk,
            "cb2": cb2,
            "wz": wz,
            "dw0": np.ascontiguousarray((W[0, 0] * delta_g[sl])[None, :]),
            "stf": np.ascontiguousarray(st[sl, None].astype(np.float32)),
            "iday": iday,
            "bb": bbuf,
        })
    return in_maps


def kernel(**inputs) -> np.ndarray:
    nc = _get_nc()
    in_maps = _prep_inputs(**inputs)
    res = run_bass_kernel_spmd(nc, in_maps, list(range(NCORES))).results
    out = np.concatenate([res[c]["out"] for c in range(NCORES)], axis=0)
    return out.astype(np.float32)


if __name__ == "__main__":
    import jax
    import reference
    cpu = jax.devices("cpu")[0]
    with jax.default_device(cpu):
        ins = {k: np.asarray(v) for k, v in reference.setup_inputs().items()}
        exp = np.asarray(reference.reference(**reference.setup_inputs()))
    got = kernel(**ins)
    err = np.abs(got - exp).max()
    rel = err / max(np.abs(exp).max(), 1e-9)
    print("max abs err:", err, "rel:", rel)
